# revision 41
# baseline (speedup 1.0000x reference)
"""Trainium2 Bass kernel: causal sliding-window attention block.

Model (see reference): x:[2,2048,512] -> q/k/v proj (8 heads x 64) ->
causal sliding-window attention (W=128) -> out proj.

Sharding: 8 cores = 2 batches x 4 sequence chunks of 512 rows.
Each core gets a 640-row halo slice of x (transposed, bf16), all four
weight matrices (bf16), and computes its 512x512 f32 output chunk.
No cross-core communication; the host concatenates chunks.

Per-core kernel (bf16 matmuls, fp32 PSUM accumulation, rel err ~5e-3):
  qT = Wq^T x^T   [512,512]  stored twice with the other head's rows
                  zeroed (score matmuls then contract K=128 from base
                  partition 0 -- row-group-packed K=64 matmuls with
                  base-64 operands lock up TRN2)
  kT = Wk^T x_halo^T [512,640];  v = x_halo Wv [640,512] natural
  per (head-pair, halo k-block): scoresT[w,p] = k_blk q^T (transposed
    scores: softmax stats and A@V need no on-chip transposes)
  expT = exp(scoresT/8) * maskT  (ACT exp from PSUM; triangular
    {0,1} masks multiplied on GPSIMD)
  sums[h,p] += onehot_h^T expT   (column sums via PE matmuls into a
    per-pair [2,512] PSUM tile; zero-fill matmul opens the
    accumulation group -- an intermediate stop_tensor_calc clears
    has_written and would turn accumulate into overwrite)
  oT[dh,p] += v_blk^T expT       (col-group-packed head pair,
    overlapping-range PSUM accumulation)
  oT_norm = oT * (1/sums)        (reciprocal + DRAM-bounce broadcast
    of 1/sums down each head's 64 rows; SBUF partition broadcast is
    not expressible)
  out = oT_norm^T Wo  [512,512] -> bf16 out, host upcasts
"""

from contextlib import ExitStack

import numpy as np
import ml_dtypes

import concourse.bacc as bacc
import concourse.tile as tile
import concourse.mybir as mybir
import concourse.bass as bass
from concourse import bass_utils

BF16 = mybir.dt.bfloat16
F32 = mybir.dt.float32

P = 128          # partitions / block size / window
S = 512          # chunk rows per core
SH = 640         # halo rows per core (128 + 512)
D = 512          # d_model
H = 8            # heads
DH = 64          # head dim
NKT = 4          # d_model tiles of 128
NST = 5          # halo row tiles of 128
NPAIR = 4        # head pairs
NHB = 5          # halo k-blocks
N_CORES = 8

_nc_cache = None


def _build_kernel():
    nc = bacc.Bacc("TRN2", target_bir_lowering=False, debug=False,
                   enable_asserts=False)

    xT_d = nc.dram_tensor("xT", [D, SH], BF16, kind="ExternalInput")
    wq_d = nc.dram_tensor("wq", [D, D], BF16, kind="ExternalInput")
    wk_d = nc.dram_tensor("wk", [D, D], BF16, kind="ExternalInput")
    wv_d = nc.dram_tensor("wv", [D, D], BF16, kind="ExternalInput")
    wo_d = nc.dram_tensor("wo", [D, D], BF16, kind="ExternalInput")
    masks_d = nc.dram_tensor("masks", [P, 1028], BF16, kind="ExternalInput")
    out_d = nc.dram_tensor("out", [S, D], BF16, kind="ExternalOutput")

    with tile.TileContext(nc) as tc, ExitStack() as ctx:
        _kernel_body(ctx, tc, xT_d, wq_d, wk_d, wv_d, wo_d, masks_d, out_d)
    nc.compile()
    return nc


def _kernel_body(ctx, tc, xT_d, wq_d, wk_d, wv_d, wo_d, masks_d, out_d):
    nc = tc.nc
    Exp = mybir.ActivationFunctionType.Exp

    persist = ctx.enter_context(tc.tile_pool(name="persist", bufs=1))
    expp = ctx.enter_context(tc.tile_pool(name="expp", bufs=3))
    rbcp = ctx.enter_context(tc.tile_pool(name="rbcp", bufs=2))
    dramp = ctx.enter_context(tc.tile_pool(name="dramp", bufs=1, space="DRAM"))
    pp512 = ctx.enter_context(tc.tile_pool(name="pp512", bufs=2, space="PSUM"))
    psc = ctx.enter_context(tc.tile_pool(name="psc", bufs=2, space="PSUM"))
    poT = ctx.enter_context(tc.tile_pool(name="poT", bufs=2, space="PSUM"))
    psums = ctx.enter_context(tc.tile_pool(name="psums", bufs=2, space="PSUM"))

    # memsets first: zero-fill matmuls then have no input dependencies
    zeros1 = persist.tile([1, 512], BF16, tag="zeros1")
    nc.gpsimd.memset(zeros1[:], 0.0)
    sel_even = persist.tile([P, 1], F32, tag="sel_even")
    nc.gpsimd.memset(sel_even[:], 1.0)
    nc.gpsimd.memset(sel_even[DH:P, :], 0.0)
    sel_odd = persist.tile([P, 1], F32, tag="sel_odd")
    nc.gpsimd.memset(sel_odd[:], 0.0)
    nc.gpsimd.memset(sel_odd[DH:P, :], 1.0)

    # ---------------- load inputs ----------------
    # xT split per k-tile so the first projection matmuls start early;
    # weights one DMA each (SP dispatch is costly). wq first.
    xT_big = persist.tile([P, NKT * SH], BF16, tag="xT")
    xT = [xT_big[:, k * SH:(k + 1) * SH] for k in range(NKT)]
    nc.sync.dma_start(xT[0][:], xT_d.ap()[0:P, :])

    wqbig = persist.tile([P, NKT * D], BF16, tag="wq", name="wqbig")
    wq_sb = [wqbig[:, k * D:(k + 1) * D] for k in range(NKT)]
    nc.sync.dma_start(wq_sb[0][:], wq_d.ap()[0:P, :])
    nc.sync.dma_start(
        wqbig[:, D:NKT * D].rearrange("p (k d) -> p k d", k=NKT - 1),
        wq_d.ap()[P:, :].rearrange("(k p) d -> p k d", p=P))

    def load_w_on(eng, name, dram):
        big = persist.tile([P, NKT * D], BF16, tag=name, name=f"{name}big")
        eng.dma_start(big[:].rearrange("p (k d) -> p k d", k=NKT),
                      dram.ap().rearrange("(k p) d -> p k d", p=P))
        return [big[:, k * D:(k + 1) * D] for k in range(NKT)]

    for k in range(1, NKT):
        nc.sync.dma_start(xT[k][:], xT_d.ap()[k * P:(k + 1) * P, :])
    wk_sb = load_w_on(nc.sync, "wk", wk_d)
    wv_sb = load_w_on(nc.sync, "wv", wv_d)
    wo_sb = load_w_on(nc.scalar, "wo", wo_d)

    masks = persist.tile([P, 1028], BF16, tag="masks")
    nc.sync.dma_start(masks[:], masks_d.ap()[:, :])
    mask_mid = masks[:, 0:512]
    mask_e0 = masks[:, 512:768]
    mask_e4 = masks[:, 768:1024]
    onehots = masks[:, 1024:1028]

    # early zero-fill of the first two pairs' accumulators: PE work with no
    # DMA dependencies, fills the input-load stall
    def alloc_pair_psum():
        sp = psums.tile([2, 512], F32, tag="sums", name="sums")
        nc.tensor.matmul(sp[:], zeros1[0:1, 0:2], zeros1[0:1, 0:512],
                         start=True, stop=False, skip_group_check=True)
        op = poT.tile([P, 512], F32, tag="oT", name="oTps")
        nc.tensor.matmul(op[:], zeros1[0:1, 0:P], zeros1[0:1, 0:512],
                         start=True, stop=False, skip_group_check=True)
        return sp, op

    pair_psum = {t: alloc_pair_psum() for t in (0, 1)}

    # ---------------- projections ----------------
    # qT[m] = sum_k Wq[k,m]^T @ xT[k][:, 128:640]  -> [128, 512] (chunk cols)
    # Stored twice with the other head's rows zeroed, so score matmuls can
    # contract K=128 from base partition 0 (row-group packed K=64 matmuls
    # at base 64 lock up the device).
    qTe_sb, qTo_sb = [], []
    Copy = mybir.ActivationFunctionType.Copy
    for m in range(NKT):
        ps = pp512.tile([P, 512], F32, tag="ps512", name="ps512")
        for k in range(NKT):
            nc.tensor.matmul(ps[:], wq_sb[k][:, m * P:(m + 1) * P],
                             xT[k][:, P:SH], start=(k == 0), stop=(k == NKT - 1))
        te = persist.tile([P, 512], BF16, tag=f"qTe{m}", name=f"qTe{m}")
        nc.scalar.activation(te[:], ps[:], Copy, scale=sel_even[:])
        qTe_sb.append(te)
        to = persist.tile([P, 512], BF16, tag=f"qTo{m}", name=f"qTo{m}")
        nc.vector.tensor_scalar_mul(to[:], ps[:], sel_odd[:])
        qTo_sb.append(to)

    # kT[m] = sum_k Wk[k,m]^T @ xT[k]  -> [128, 640] (halo cols)
    kT_sb = []
    for m in range(NKT):
        t = persist.tile([P, SH], BF16, tag=f"kT{m}", name=f"kT{m}")
        ps = pp512.tile([P, 512], F32, tag="ps512", name="ps512")
        for k in range(NKT):
            nc.tensor.matmul(ps[:], wk_sb[k][:, m * P:(m + 1) * P],
                             xT[k][:, 0:512], start=(k == 0), stop=(k == NKT - 1))
        nc.scalar.copy(t[:, 0:512], ps[:])
        ps2 = pp512.tile([P, 512], F32, tag="ps512", name="ps512")
        for k in range(NKT):
            nc.tensor.matmul(ps2[:, 0:P], wk_sb[k][:, m * P:(m + 1) * P],
                             xT[k][:, 512:SH], start=(k == 0), stop=(k == NKT - 1))
        nc.vector.tensor_copy(t[:, 512:SH], ps2[:, 0:P])
        kT_sb.append(t)

    # v[st] = sum_k xT[k][:, st]^T @ Wv[k]  -> [128, 512] natural rows
    v_sb = []
    for st in range(NST):
        ps = pp512.tile([P, 512], F32, tag="ps512", name="ps512")
        for k in range(NKT):
            nc.tensor.matmul(ps[:], xT[k][:, st * P:(st + 1) * P],
                             wv_sb[k][:], start=(k == 0), stop=(k == NKT - 1))
        t = persist.tile([P, 512], BF16, tag=f"v{st}", name=f"v{st}")
        nc.vector.tensor_copy(t[:], ps[:])
        v_sb.append(t)

    # ---------------- attention ----------------
    rinv_sb = persist.tile([2, 512], F32, tag="rinv")
    oT_sb = []

    for t in range(NPAIR):
        h0, h1 = 2 * t, 2 * t + 1
        sums_ps, oT_ps = pair_psum[t] if t in pair_psum else alloc_pair_psum()
        for hb in range(NHB):
            pc0 = max(0, (hb - 1) * P)
            pc1 = min(512, (hb + 1) * P)
            pw = pc1 - pc0
            sc = psc.tile([P, 512], F32, tag="sc", name="sc")
            # scores^T for both heads; K=128 with the other head's q zeroed
            nc.tensor.matmul(sc[:, 0:pw],
                             kT_sb[t][:, hb * P:(hb + 1) * P],
                             qTe_sb[t][:, pc0:pc1],
                             start=True, stop=True)
            nc.tensor.matmul(sc[:, pw:2 * pw],
                             kT_sb[t][:, hb * P:(hb + 1) * P],
                             qTo_sb[t][:, pc0:pc1],
                             start=True, stop=True)
            # exp( scores / 8 )
            et = expp.tile([P, 512], BF16, tag="expT", name="expT")
            nc.scalar.activation(et[:, 0:2 * pw], sc[:, 0:2 * pw], Exp,
                                 scale=0.125)
            # multiplicative triangular mask
            if hb == 0:
                mask = mask_e0
            elif hb == NHB - 1:
                mask = mask_e4
            else:
                mask = mask_mid
            nc.gpsimd.tensor_mul(et[:, 0:2 * pw], et[:, 0:2 * pw],
                                 mask[:, 0:2 * pw])
            # column sums via one-hot matmuls (accumulate into sums_ps)
            last = (hb == NHB - 1)
            nc.tensor.matmul(sums_ps[:, pc0:pc1],
                             onehots[:, 0:2], et[:, 0:pw],
                             start=False, stop=False, skip_group_check=True)
            nc.tensor.matmul(sums_ps[:, pc0:pc1],
                             onehots[:, 2:4], et[:, pw:2 * pw],
                             start=False, stop=last, skip_group_check=True)
            # attn @ v: transposed head outputs, col-group packed pair,
            # overlapping-range accumulation (zero-fill MM cleared the bank)
            nc.tensor.matmul(oT_ps[0:DH, pc0:pc1],
                             v_sb[hb][:, h0 * DH:(h0 + 1) * DH], et[:, 0:pw],
                             start=False, stop=last, tile_position=(0, 0),
                             skip_group_check=True)
            nc.tensor.matmul(oT_ps[DH:P, pc0:pc1],
                             v_sb[hb][:, h1 * DH:(h1 + 1) * DH], et[:, pw:2 * pw],
                             start=False, stop=last, tile_position=(0, 64),
                             skip_group_check=True)
        # normalize: oT * (1/sums) broadcast down each head's 64 rows
        nc.vector.reciprocal(rinv_sb[:], sums_ps[:])
        rdram = dramp.tile([2, 512], F32, tag="rdram", name="rdram", bufs=2)
        nc.sync.dma_start(rdram[:], rinv_sb[:])
        rbc = rbcp.tile([P, 512], F32, tag="rbc", name="rbc")
        nc.sync.dma_start(rbc[0:DH, :],
                          rdram[0:1, :].to_broadcast((DH, 512)))
        nc.sync.dma_start(rbc[DH:P, :],
                          rdram[1:2, :].to_broadcast((DH, 512)))
        ot = persist.tile([P, 512], BF16, tag=f"oT{t}", name=f"oTsb{t}")
        nc.vector.tensor_mul(ot[:], oT_ps[:], rbc[:])
        oT_sb.append(ot)

    # ---------------- output projection ----------------
    # m-tiles 0,1 accumulate pairs 0-2 while pair 3's attention finishes;
    # only their t=3 matmuls plus m-tiles 2,3 trail the last pair
    def fmm(f, mt, t):
        nc.tensor.matmul(f[:], oT_sb[t][:, mt * P:(mt + 1) * P],
                         wo_sb[t][:], start=(t == 0), stop=(t == NPAIR - 1))

    def fout(f, mt):
        osb = rbcp.tile([P, 512], BF16, tag="osb", name="osb")
        if mt % 2 == 0:
            nc.vector.tensor_copy(osb[:], f[:])
        else:
            nc.scalar.copy(osb[:], f[:])
        eng = nc.scalar if mt % 2 == 0 else nc.sync
        eng.dma_start(out_d.ap()[mt * P:(mt + 1) * P, :], osb[:])

    fps01 = [pp512.tile([P, 512], F32, tag="ps512", name=f"fps{i}")
             for i in range(2)]
    for mt in (0, 1):
        for t in (0, 1, 2):
            fmm(fps01[mt], mt, t)
    for mt in (0, 1):
        fmm(fps01[mt], mt, 3)
    for mt in (0, 1):
        fout(fps01[mt], mt)
    for mt in (2, 3):
        f = pp512.tile([P, 512], F32, tag="ps512", name="fps")
        for t in range(NPAIR):
            fmm(f, mt, t)
        fout(f, mt)


def _get_nc():
    global _nc_cache
    if _nc_cache is None:
        _nc_cache = _build_kernel()
    return _nc_cache


def _make_masks():
    j = np.arange(P)[:, None]
    c = np.arange(P)[None, :]
    curr = (j <= c).astype(ml_dtypes.bfloat16)   # k-block == q-block
    prev = (j > c).astype(ml_dtypes.bfloat16)    # k-block == q-block - 1
    mask_mid = np.concatenate([curr, prev, curr, prev], axis=1)
    mask_e0 = np.concatenate([prev, prev], axis=1)
    mask_e4 = np.concatenate([curr, curr], axis=1)
    onehots = np.zeros((P, 4), dtype=ml_dtypes.bfloat16)
    onehots[:, 0] = 1.0
    onehots[:, 3] = 1.0
    return mask_mid, mask_e0, mask_e4, onehots


def _prep_inputs(x, Wq, Wk, Wv, Wo):
    x = np.asarray(x, dtype=np.float32)
    B, S_full, _ = x.shape
    mask_mid, mask_e0, mask_e4, onehots = _make_masks()
    masks_all = np.concatenate([mask_mid, mask_e0, mask_e4, onehots], axis=1)
    masks_z = np.concatenate([mask_mid, np.zeros_like(mask_e0), mask_e4,
                              onehots], axis=1)
    wq = np.ascontiguousarray(np.asarray(Wq, np.float32).astype(ml_dtypes.bfloat16))
    wk = np.ascontiguousarray(np.asarray(Wk, np.float32).astype(ml_dtypes.bfloat16))
    wv = np.ascontiguousarray(np.asarray(Wv, np.float32).astype(ml_dtypes.bfloat16))
    wo = np.ascontiguousarray(np.asarray(Wo, np.float32).astype(ml_dtypes.bfloat16))
    in_maps = []
    for core in range(N_CORES):
        b, chunk = divmod(core, 4)
        c0 = chunk * S
        xh = np.zeros((SH, D), np.float32)
        lo = c0 - P
        src_lo = max(0, lo)
        xh[src_lo - lo:, :] = x[b, src_lo:c0 + S, :]
        xTh = np.ascontiguousarray(xh.T.astype(ml_dtypes.bfloat16))
        in_maps.append({
            "xT": xTh, "wq": wq, "wk": wk, "wv": wv, "wo": wo,
            "masks": masks_z if chunk == 0 else masks_all,
        })
    return in_maps


def kernel(x, Wq, Wk, Wv, Wo, _profile=None):
    nc = _get_nc()
    in_maps = _prep_inputs(x, Wq, Wk, Wv, Wo)
    res = bass_utils.run_bass_kernel_spmd(nc, in_maps,
                                          core_ids=list(range(N_CORES)))
    x = np.asarray(x)
    B, S_full, _ = x.shape
    out = np.empty((B, S_full, D), np.float32)
    for core in range(N_CORES):
        b, chunk = divmod(core, 4)
        out[b, chunk * S:(chunk + 1) * S, :] = (
            res.results[core]["out"].astype(np.float32))
    if _profile is not None:
        _profile.append(res)
    return out


# revision 42
# speedup vs baseline: 1.0215x; 1.0215x over previous
"""Trainium2 Bass kernel: causal sliding-window attention block.

Model (see reference): x:[2,2048,512] -> q/k/v proj (8 heads x 64) ->
causal sliding-window attention (W=128) -> out proj.

Sharding: 8 cores = 2 batches x 4 sequence chunks of 512 rows.
Each core gets a 640-row halo slice of x (transposed, bf16), all four
weight matrices (bf16), and computes its 512x512 f32 output chunk.
No cross-core communication; the host concatenates chunks.

Per-core kernel (bf16 matmuls, fp32 PSUM accumulation, rel err ~5e-3):
  qT = Wq^T x^T   [512,512]  stored twice with the other head's rows
                  zeroed (score matmuls then contract K=128 from base
                  partition 0 -- row-group-packed K=64 matmuls with
                  base-64 operands lock up TRN2)
  kT = Wk^T x_halo^T [512,640];  v = x_halo Wv [640,512] natural
  per (head-pair, halo k-block): scoresT[w,p] = k_blk q^T (transposed
    scores: softmax stats and A@V need no on-chip transposes)
  expT = exp(scoresT/8) * maskT  (ACT exp from PSUM; triangular
    {0,1} masks multiplied on GPSIMD)
  sums[h,p] += onehot_h^T expT   (column sums via PE matmuls into a
    per-pair [2,512] PSUM tile; zero-fill matmul opens the
    accumulation group -- an intermediate stop_tensor_calc clears
    has_written and would turn accumulate into overwrite)
  oT[dh,p] += v_blk^T expT       (col-group-packed head pair,
    overlapping-range PSUM accumulation)
  oT_norm = oT * (1/sums)        (reciprocal + DRAM-bounce broadcast
    of 1/sums down each head's 64 rows; SBUF partition broadcast is
    not expressible)
  out = oT_norm^T Wo  [512,512] -> bf16 out, host upcasts
"""

from contextlib import ExitStack

import numpy as np
import ml_dtypes

import concourse.bacc as bacc
import concourse.tile as tile
import concourse.mybir as mybir
import concourse.bass as bass
from concourse import bass_utils

BF16 = mybir.dt.bfloat16
F32 = mybir.dt.float32

P = 128          # partitions / block size / window
S = 512          # chunk rows per core
SH = 640         # halo rows per core (128 + 512)
D = 512          # d_model
H = 8            # heads
DH = 64          # head dim
NKT = 4          # d_model tiles of 128
NST = 5          # halo row tiles of 128
NPAIR = 4        # head pairs
NHB = 5          # halo k-blocks
N_CORES = 8

_nc_cache = None


def _build_kernel():
    nc = bacc.Bacc("TRN2", target_bir_lowering=False, debug=False,
                   enable_asserts=False)

    xT_d = nc.dram_tensor("xT", [D, SH], BF16, kind="ExternalInput")
    wq_d = nc.dram_tensor("wq", [D, D], BF16, kind="ExternalInput")
    wk_d = nc.dram_tensor("wk", [D, D], BF16, kind="ExternalInput")
    wv_d = nc.dram_tensor("wv", [D, D], BF16, kind="ExternalInput")
    wo_d = nc.dram_tensor("wo", [D, D], BF16, kind="ExternalInput")
    masks_d = nc.dram_tensor("masks", [P, 1028], BF16, kind="ExternalInput")
    out_d = nc.dram_tensor("out", [S, D], BF16, kind="ExternalOutput")

    with tile.TileContext(nc) as tc, ExitStack() as ctx:
        _kernel_body(ctx, tc, xT_d, wq_d, wk_d, wv_d, wo_d, masks_d, out_d)
    nc.compile()
    return nc


def _kernel_body(ctx, tc, xT_d, wq_d, wk_d, wv_d, wo_d, masks_d, out_d):
    nc = tc.nc
    Exp = mybir.ActivationFunctionType.Exp

    persist = ctx.enter_context(tc.tile_pool(name="persist", bufs=1))
    expp = ctx.enter_context(tc.tile_pool(name="expp", bufs=3))
    rbcp = ctx.enter_context(tc.tile_pool(name="rbcp", bufs=2))
    dramp = ctx.enter_context(tc.tile_pool(name="dramp", bufs=1, space="DRAM"))
    pp512 = ctx.enter_context(tc.tile_pool(name="pp512", bufs=2, space="PSUM"))
    psc = ctx.enter_context(tc.tile_pool(name="psc", bufs=2, space="PSUM"))
    poT = ctx.enter_context(tc.tile_pool(name="poT", bufs=2, space="PSUM"))
    psums = ctx.enter_context(tc.tile_pool(name="psums", bufs=2, space="PSUM"))

    # memsets first: zero-fill matmuls then have no input dependencies
    zeros1 = persist.tile([1, 512], BF16, tag="zeros1")
    nc.gpsimd.memset(zeros1[:], 0.0)
    sel_even = persist.tile([P, 1], F32, tag="sel_even")
    nc.gpsimd.memset(sel_even[:], 1.0)
    nc.gpsimd.memset(sel_even[DH:P, :], 0.0)
    sel_odd = persist.tile([P, 1], F32, tag="sel_odd")
    nc.gpsimd.memset(sel_odd[:], 0.0)
    nc.gpsimd.memset(sel_odd[DH:P, :], 1.0)

    # ---------------- load inputs ----------------
    # xT split per k-tile so the first projection matmuls start early;
    # weights one DMA each (SP dispatch is costly). wq first.
    xT_big = persist.tile([P, NKT * SH], BF16, tag="xT")
    xT = [xT_big[:, k * SH:(k + 1) * SH] for k in range(NKT)]
    nc.sync.dma_start(xT[0][:], xT_d.ap()[0:P, :])

    wqbig = persist.tile([P, NKT * D], BF16, tag="wq", name="wqbig")
    wq_sb = [wqbig[:, k * D:(k + 1) * D] for k in range(NKT)]
    nc.sync.dma_start(wq_sb[0][:], wq_d.ap()[0:P, :])
    nc.sync.dma_start(
        wqbig[:, D:NKT * D].rearrange("p (k d) -> p k d", k=NKT - 1),
        wq_d.ap()[P:, :].rearrange("(k p) d -> p k d", p=P))

    def load_w_on(eng, name, dram):
        big = persist.tile([P, NKT * D], BF16, tag=name, name=f"{name}big")
        eng.dma_start(big[:].rearrange("p (k d) -> p k d", k=NKT),
                      dram.ap().rearrange("(k p) d -> p k d", p=P))
        return [big[:, k * D:(k + 1) * D] for k in range(NKT)]

    for k in range(1, NKT):
        nc.sync.dma_start(xT[k][:], xT_d.ap()[k * P:(k + 1) * P, :])
    wk_sb = load_w_on(nc.sync, "wk", wk_d)
    wv_sb = load_w_on(nc.sync, "wv", wv_d)
    wo_sb = load_w_on(nc.scalar, "wo", wo_d)

    masks = persist.tile([P, 1028], BF16, tag="masks")
    nc.sync.dma_start(masks[:], masks_d.ap()[:, :])
    mask_mid = masks[:, 0:512]
    mask_e0 = masks[:, 512:768]
    mask_e4 = masks[:, 768:1024]
    onehots = masks[:, 1024:1028]

    # early zero-fill of the first two pairs' accumulators: PE work with no
    # DMA dependencies, fills the input-load stall
    def alloc_pair_psum():
        sp = psums.tile([2, 512], F32, tag="sums", name="sums")
        nc.tensor.matmul(sp[:], zeros1[0:1, 0:2], zeros1[0:1, 0:512],
                         start=True, stop=False, skip_group_check=True)
        op = poT.tile([P, 512], F32, tag="oT", name="oTps")
        nc.tensor.matmul(op[:], zeros1[0:1, 0:P], zeros1[0:1, 0:512],
                         start=True, stop=False, skip_group_check=True)
        return sp, op

    pair_psum = {t: alloc_pair_psum() for t in (0, 1)}

    # ---------------- projections ----------------
    # qT[m] = sum_k Wq[k,m]^T @ xT[k][:, 128:640]  -> [128, 512] (chunk cols)
    # Stored twice with the other head's rows zeroed, so score matmuls can
    # contract K=128 from base partition 0 (row-group packed K=64 matmuls
    # at base 64 lock up the device).
    qTe_sb, qTo_sb = [], []
    Copy = mybir.ActivationFunctionType.Copy
    for m in range(NKT):
        ps = pp512.tile([P, 512], F32, tag="ps512", name="ps512")
        for k in range(NKT):
            nc.tensor.matmul(ps[:], wq_sb[k][:, m * P:(m + 1) * P],
                             xT[k][:, P:SH], start=(k == 0), stop=(k == NKT - 1))
        te = persist.tile([P, 512], BF16, tag=f"qTe{m}", name=f"qTe{m}")
        nc.scalar.activation(te[:], ps[:], Copy, scale=sel_even[:])
        qTe_sb.append(te)
        to = persist.tile([P, 512], BF16, tag=f"qTo{m}", name=f"qTo{m}")
        nc.vector.tensor_scalar_mul(to[:], ps[:], sel_odd[:])
        qTo_sb.append(to)

    # kT[m] = sum_k Wk[k,m]^T @ xT[k]  -> [128, 640] (halo cols)
    kT_sb = []
    for m in range(NKT):
        t = persist.tile([P, SH], BF16, tag=f"kT{m}", name=f"kT{m}")
        ps = pp512.tile([P, 512], F32, tag="ps512", name="ps512")
        for k in range(NKT):
            nc.tensor.matmul(ps[:], wk_sb[k][:, m * P:(m + 1) * P],
                             xT[k][:, 0:512], start=(k == 0), stop=(k == NKT - 1))
        nc.scalar.copy(t[:, 0:512], ps[:])
        ps2 = pp512.tile([P, 512], F32, tag="ps512", name="ps512")
        for k in range(NKT):
            nc.tensor.matmul(ps2[:, 0:P], wk_sb[k][:, m * P:(m + 1) * P],
                             xT[k][:, 512:SH], start=(k == 0), stop=(k == NKT - 1))
        nc.vector.tensor_copy(t[:, 512:SH], ps2[:, 0:P])
        kT_sb.append(t)

    # v[st] = sum_k xT[k][:, st]^T @ Wv[k]  -> [128, 512] natural rows
    v_sb = []
    for st in range(NST):
        ps = pp512.tile([P, 512], F32, tag="ps512", name="ps512")
        for k in range(NKT):
            nc.tensor.matmul(ps[:], xT[k][:, st * P:(st + 1) * P],
                             wv_sb[k][:], start=(k == 0), stop=(k == NKT - 1))
        t = persist.tile([P, 512], BF16, tag=f"v{st}", name=f"v{st}")
        nc.vector.tensor_copy(t[:], ps[:])
        v_sb.append(t)

    # ---------------- attention ----------------
    rinv_sb = persist.tile([2, 512], F32, tag="rinv")
    oT_sb = []

    for t in range(NPAIR):
        h0, h1 = 2 * t, 2 * t + 1
        sums_ps, oT_ps = pair_psum[t] if t in pair_psum else alloc_pair_psum()
        for hb in range(NHB):
            pc0 = max(0, (hb - 1) * P)
            pc1 = min(512, (hb + 1) * P)
            pw = pc1 - pc0
            sc = psc.tile([P, 512], F32, tag="sc", name="sc")
            # scores^T for both heads; K=128 with the other head's q zeroed
            nc.tensor.matmul(sc[:, 0:pw],
                             kT_sb[t][:, hb * P:(hb + 1) * P],
                             qTe_sb[t][:, pc0:pc1],
                             start=True, stop=True)
            nc.tensor.matmul(sc[:, pw:2 * pw],
                             kT_sb[t][:, hb * P:(hb + 1) * P],
                             qTo_sb[t][:, pc0:pc1],
                             start=True, stop=True)
            # exp( scores / 8 )
            et = expp.tile([P, 512], BF16, tag="expT", name="expT")
            nc.scalar.activation(et[:, 0:2 * pw], sc[:, 0:2 * pw], Exp,
                                 scale=0.125)
            # multiplicative triangular mask
            if hb == 0:
                mask = mask_e0
            elif hb == NHB - 1:
                mask = mask_e4
            else:
                mask = mask_mid
            nc.gpsimd.tensor_mul(et[:, 0:2 * pw], et[:, 0:2 * pw],
                                 mask[:, 0:2 * pw])
            # column sums via one-hot matmuls (accumulate into sums_ps)
            last = (hb == NHB - 1)
            nc.tensor.matmul(sums_ps[:, pc0:pc1],
                             onehots[:, 0:2], et[:, 0:pw],
                             start=False, stop=False, skip_group_check=True)
            nc.tensor.matmul(sums_ps[:, pc0:pc1],
                             onehots[:, 2:4], et[:, pw:2 * pw],
                             start=False, stop=last, skip_group_check=True)
            # attn @ v: transposed head outputs, col-group packed pair,
            # overlapping-range accumulation (zero-fill MM cleared the bank)
            nc.tensor.matmul(oT_ps[0:DH, pc0:pc1],
                             v_sb[hb][:, h0 * DH:(h0 + 1) * DH], et[:, 0:pw],
                             start=False, stop=last, tile_position=(0, 0),
                             skip_group_check=True)
            nc.tensor.matmul(oT_ps[DH:P, pc0:pc1],
                             v_sb[hb][:, h1 * DH:(h1 + 1) * DH], et[:, pw:2 * pw],
                             start=False, stop=last, tile_position=(0, 64),
                             skip_group_check=True)
        # normalize: oT * (1/sums) broadcast down each head's 64 rows
        nc.vector.reciprocal(rinv_sb[:], sums_ps[:])
        rdram = dramp.tile([2, 512], F32, tag="rdram", name="rdram", bufs=2)
        nc.sync.dma_start(rdram[:], rinv_sb[:])
        rbc = rbcp.tile([P, 512], F32, tag="rbc", name="rbc")
        nc.sync.dma_start(rbc[0:DH, :],
                          rdram[0:1, :].to_broadcast((DH, 512)))
        nc.sync.dma_start(rbc[DH:P, :],
                          rdram[1:2, :].to_broadcast((DH, 512)))
        ot = persist.tile([P, 512], BF16, tag=f"oT{t}", name=f"oTsb{t}")
        nc.vector.tensor_mul(ot[:], oT_ps[:], rbc[:])
        oT_sb.append(ot)

    # ---------------- output projection ----------------
    # m-tiles 0,1 accumulate pairs 0-2 while pair 3's attention finishes;
    # only their t=3 matmuls plus m-tiles 2,3 trail the last pair
    def fmm(f, mt, t):
        nc.tensor.matmul(f[:], oT_sb[t][:, mt * P:(mt + 1) * P],
                         wo_sb[t][:], start=(t == 0), stop=(t == NPAIR - 1))

    def fout(f, mt):
        osb = rbcp.tile([P, 512], BF16, tag="osb", name="osb")
        if mt % 2 == 0:
            nc.vector.tensor_copy(osb[:], f[:])
        else:
            nc.scalar.copy(osb[:], f[:])
        eng = nc.scalar if mt % 2 == 0 else nc.sync
        eng.dma_start(out_d.ap()[mt * P:(mt + 1) * P, :], osb[:])

    # four concurrent accumulators: m0/m1 from pp512, m2/m3 borrow the
    # sums/oT pool slots that pairs 0/1 released mid-attention — so all
    # twelve t<=2 matmuls run while pair 3's normalize bounce is in flight
    fps = [pp512.tile([P, 512], F32, tag="ps512", name="fps0"),
           pp512.tile([P, 512], F32, tag="ps512", name="fps1"),
           psums.tile([P, 512], F32, tag="sums", name="fps2"),
           poT.tile([P, 512], F32, tag="oT", name="fps3")]
    for t in (0, 1, 2):
        for mt in range(NKT):
            fmm(fps[mt], mt, t)
    for mt in range(NKT):
        fmm(fps[mt], mt, 3)
    for mt in range(NKT):
        fout(fps[mt], mt)


def _get_nc():
    global _nc_cache
    if _nc_cache is None:
        _nc_cache = _build_kernel()
    return _nc_cache


def _make_masks():
    j = np.arange(P)[:, None]
    c = np.arange(P)[None, :]
    curr = (j <= c).astype(ml_dtypes.bfloat16)   # k-block == q-block
    prev = (j > c).astype(ml_dtypes.bfloat16)    # k-block == q-block - 1
    mask_mid = np.concatenate([curr, prev, curr, prev], axis=1)
    mask_e0 = np.concatenate([prev, prev], axis=1)
    mask_e4 = np.concatenate([curr, curr], axis=1)
    onehots = np.zeros((P, 4), dtype=ml_dtypes.bfloat16)
    onehots[:, 0] = 1.0
    onehots[:, 3] = 1.0
    return mask_mid, mask_e0, mask_e4, onehots


def _prep_inputs(x, Wq, Wk, Wv, Wo):
    x = np.asarray(x, dtype=np.float32)
    B, S_full, _ = x.shape
    mask_mid, mask_e0, mask_e4, onehots = _make_masks()
    masks_all = np.concatenate([mask_mid, mask_e0, mask_e4, onehots], axis=1)
    masks_z = np.concatenate([mask_mid, np.zeros_like(mask_e0), mask_e4,
                              onehots], axis=1)
    wq = np.ascontiguousarray(np.asarray(Wq, np.float32).astype(ml_dtypes.bfloat16))
    wk = np.ascontiguousarray(np.asarray(Wk, np.float32).astype(ml_dtypes.bfloat16))
    wv = np.ascontiguousarray(np.asarray(Wv, np.float32).astype(ml_dtypes.bfloat16))
    wo = np.ascontiguousarray(np.asarray(Wo, np.float32).astype(ml_dtypes.bfloat16))
    in_maps = []
    for core in range(N_CORES):
        b, chunk = divmod(core, 4)
        c0 = chunk * S
        xh = np.zeros((SH, D), np.float32)
        lo = c0 - P
        src_lo = max(0, lo)
        xh[src_lo - lo:, :] = x[b, src_lo:c0 + S, :]
        xTh = np.ascontiguousarray(xh.T.astype(ml_dtypes.bfloat16))
        in_maps.append({
            "xT": xTh, "wq": wq, "wk": wk, "wv": wv, "wo": wo,
            "masks": masks_z if chunk == 0 else masks_all,
        })
    return in_maps


def kernel(x, Wq, Wk, Wv, Wo, _profile=None):
    nc = _get_nc()
    in_maps = _prep_inputs(x, Wq, Wk, Wv, Wo)
    res = bass_utils.run_bass_kernel_spmd(nc, in_maps,
                                          core_ids=list(range(N_CORES)))
    x = np.asarray(x)
    B, S_full, _ = x.shape
    out = np.empty((B, S_full, D), np.float32)
    for core in range(N_CORES):
        b, chunk = divmod(core, 4)
        out[b, chunk * S:(chunk + 1) * S, :] = (
            res.results[core]["out"].astype(np.float32))
    if _profile is not None:
        _profile.append(res)
    return out
